# revision 17
# baseline (speedup 1.0000x reference)
"""Bass/Trainium2 kernel for nn_DefaultSegmentLinear (fp8 segment linear).

Reference semantics (CHUNKS=4, seg_mode='weight'):
    xq = e4m3fn(x / in_scale)                       # OCP e4m3, max 448
    wq = e4m3fn(w_c / w_scales[c])                  # per out-chunk of 1024
    out = (xq @ wq_c^T) * in_scale * w_scales[c] + bias

Sharding: 4-way over the 16384 tokens x 2-way over the 4096 out
features (8 cores; core cid -> token quarter q=cid//2, out half
h=cid%2).

Quantization happens on the HOST: x and w are divided by their
calibration scales (exact f32 division, matching the reference), scaled
by 0.5, and rounded to TRN e4m3 (max 240) via ml_dtypes.float8_e4m3 --
every OCP-e4m3 grid point v <= 448 has v/2 <= 224 exactly representable
in TRN e4m3, and numpy's f32 multiply + RNE downcast is bit-identical
to the device ACT path the previous revision used.  The 4x is folded
into the output scale alpha_c = 4*in_scale*w_scales[c].  Shipping fp8
instead of f32 cuts device DMA-in 4x (x: 64->16 MB, w: 32->8 MB per
core) and removes the on-device quantize pass entirely.

Per-core DRAM tensors (contraction i on partitions for both operands):
    xq8  [128, G, KT, 2, TG] fp8   pre-tiled (x/in_scale/2)^T quarter,
         grouped into G=4 token groups of TG=1024 so matmuls start
         after ~one group's DMA instead of the full x load
    wq8  [128, OT, KT, 2, 128] fp8 pre-tiled (w/w_scale/2)^T half
    outT [OUT_C, T] f32  (o, t); host transposes back

Device schedule: wq stays resident in SBUF (64 KB/partition); xq token
groups double-buffer (2 x 32 KB/partition).  Per (group, o-tile): 16
k-steps x 2 token banks of DoubleRow fp8 matmuls (K=256, N=512) into
PSUM, then one DVE tensor_scalar (psum*alpha + bias) per bank and a DMA
out.  Group g+1's x DMAs are paced one k-tile per o-tile iteration of
group g; weight DMAs for ot>=2 stream behind group 0's compute.
"""

import os

import ml_dtypes
import numpy as np

import concourse.bacc as bacc
import concourse.mybir as mybir
from concourse import tile
from concourse.bass_utils import run_bass_kernel_spmd

N_CORES = 8
TOKEN_WAYS, OUT_WAYS = (
    int(v) for v in os.environ.get("TRN_KERNEL_SHARD", "4x2").split("x")
)
assert TOKEN_WAYS * OUT_WAYS == N_CORES
B, S, IN, OUT = 4, 4096, 4096, 4096
TOK = B * S
T = TOK // TOKEN_WAYS    # 4096 tokens per core
OUT_C = OUT // OUT_WAYS  # 2048 out features per core
KT = IN // 256           # 16 contraction super-tiles (256 = 128 x 2)
OT = OUT_C // 128        # 16 out-feature tiles per core
NT = 512                 # moving free dim per matmul (one PSUM bank of f32)
TG = int(os.environ.get("TRN_KERNEL_TG", "1024"))  # tokens per group
G = T // TG              # token groups per core
BG = TG // NT            # PSUM banks per (group, o-tile)
CHUNKS = 4
CHUNKS_C = CHUNKS // OUT_WAYS  # 2 weight chunks per core
OT_PER_CHUNK = OT // CHUNKS_C  # 8

F32 = mybir.dt.float32
FP8 = mybir.dt.float8e4
NP_FP8 = ml_dtypes.float8_e4m3

_CACHE = {}


def _build():
    key = ("nc", TG)
    if key in _CACHE:
        return _CACHE[key]
    nc = bacc.Bacc(None, target_bir_lowering=False)
    xq8 = nc.dram_tensor("xq8", [128, G, KT, 2, TG], FP8, kind="ExternalInput")
    wq8 = nc.dram_tensor("wq8", [128, OT, KT, 2, 128], FP8, kind="ExternalInput")
    biasv = nc.dram_tensor("biasv", [OUT_C], F32, kind="ExternalInput")
    alpha = nc.dram_tensor("alpha", [CHUNKS_C], F32, kind="ExternalInput")
    outT = nc.dram_tensor("outT", [OUT_C, T], F32, kind="ExternalOutput")

    DR = mybir.MatmulPerfMode.DoubleRow

    with tile.TileContext(nc) as tc:
        with (
            tc.tile_pool(name="consts", bufs=1) as consts,
            tc.tile_pool(name="wq", bufs=1) as wqp,
            tc.tile_pool(name="xq", bufs=2) as xqp,
            tc.tile_pool(name="osb", bufs=4) as osbp,
            tc.tile_pool(name="psum", bufs=8, space="PSUM") as psp,
        ):
            wq = wqp.tile([128, OT, KT, 2, 128], FP8, tag="wq", name="wq")

            # Warm the PE clock (HAM un-throttles after ~3.4us of
            # sustained activity) with throwaway matmuls on a memset
            # tile -- no DMA dependency, so they start at ~0.3us and
            # span the whole input-DMA window; real matmuls then start
            # at 2.4 GHz instead of paying ~13 cold issues at 1.2 GHz.
            warm_w = consts.tile([128, 2, 128], FP8, tag="warm")
            nc.vector.memset(warm_w[:], 0.0)
            ps_warm = psp.tile([128, NT], F32, tag="ps", name="ps_warm")
            for _ in range(42):
                nc.tensor.matmul(
                    ps_warm[:, :128],
                    lhsT=warm_w[:],
                    rhs=warm_w[:],
                    start=True,
                    stop=True,
                    perf_mode=DR,
                )

            # DMA emission order controls which transfers the first
            # matmuls wait on: w[0] + group 0's first 8 k-tiles of x
            # (~2.5 MB) land first; everything else streams behind.
            # Descriptor generation costs ~600ns per dma_start on the
            # issuing sequencer, so the critical transfers are split
            # across both HWDGE rings (sync + scalar) to halve the
            # serial generation latency.
            xq_cur = xqp.tile([128, KT, 2, TG], FP8, tag="xq", name="xq0")
            nc.sync.dma_start(out=wq[:, 0], in_=wq8[:, 0])
            nc.scalar.dma_start(out=wq[:, 1], in_=wq8[:, 1])
            nc.sync.dma_start(out=xq_cur[:, 0], in_=xq8[:, 0, 0])
            nc.scalar.dma_start(out=xq_cur[:, 1], in_=xq8[:, 0, 1])
            nc.scalar.dma_start(out=wq[:, 2], in_=wq8[:, 2])
            for k in range(2, KT):
                eng = nc.sync if k % 2 == 0 else nc.scalar
                eng.dma_start(out=xq_cur[:, k], in_=xq8[:, 0, k])
            al_b = []
            for c in range(CHUNKS_C):
                t2 = consts.tile([128, 1], F32, tag=f"al{c}")
                nc.sync.dma_start(
                    out=t2[:], in_=alpha[c : c + 1].to_broadcast((128, 1))
                )
                al_b.append(t2)
            bias_sb = consts.tile([128, OT], F32, tag="bias")
            nc.sync.dma_start(
                out=bias_sb[:], in_=biasv[:].rearrange("(j p) -> p j", p=128)
            )

            def mm_block(ps, ot, xq_t, k_lo, k_hi):
                for k in range(k_lo, k_hi):
                    for b in range(BG):
                        nc.tensor.matmul(
                            ps[b][:],
                            lhsT=wq[:, ot, k],
                            rhs=xq_t[:, k, :, NT * b : NT * (b + 1)],
                            start=(k == 0),
                            stop=(k == KT - 1),
                            perf_mode=DR,
                        )

            def epilogue(ps, g, ot):
                c = ot // OT_PER_CHUNK
                for b in range(BG):
                    ob = osbp.tile([128, NT], F32, tag="osb", name=f"ob{g}_{ot}_{b}")
                    nc.vector.tensor_scalar(
                        ob[:],
                        ps[b][:],
                        al_b[c][:],
                        bias_sb[:, ot : ot + 1],
                        op0=mybir.AluOpType.mult,
                        op1=mybir.AluOpType.add,
                    )
                    nc.sync.dma_start(
                        out=outT[
                            128 * ot : 128 * (ot + 1),
                            TG * g + NT * b : TG * g + NT * (b + 1),
                        ],
                        in_=ob[:],
                    )

            for g in range(G):
                xq_next = None
                if g + 1 < G:
                    xq_next = xqp.tile(
                        [128, KT, 2, TG], FP8, tag="xq", name=f"xq{g + 1}"
                    )
                nxt_k = 0   # next k-tile of xq_next to prefetch
                nxt_w = 3   # next weight o-tile to prefetch (g0 only)

                def prefetch(n_x, n_w, g=g, xq_next=xq_next):
                    # The weight stream rides the sync HWDGE ring,
                    # where FIFO order behind the (epilogue-gated)
                    # output stores paces it -- otherwise the SDMA
                    # engines' fair round-robin lets these 6.5 MB
                    # steal bandwidth from the startup-critical
                    # transfers.  Next-group x rides the scalar ring.
                    nonlocal nxt_k, nxt_w
                    for _ in range(n_w):
                        if g == 0 and nxt_w < OT:
                            nc.sync.dma_start(out=wq[:, nxt_w], in_=wq8[:, nxt_w])
                            nxt_w += 1
                    for _ in range(n_x):
                        if xq_next is not None and nxt_k < KT:
                            nc.scalar.dma_start(
                                out=xq_next[:, nxt_k], in_=xq8[:, g + 1, nxt_k]
                            )
                            nxt_k += 1

                if g == 0:
                    # Progressive-K start: the first 3 o-tiles
                    # accumulate two k-tiles per pass, so the first
                    # matmul waits on only w[0..1] + x k[0..1]
                    # (~1.25 MB) and the rest of group 0's x streams
                    # in underneath the early (DMA-bound) passes.
                    ps3 = [
                        [psp.tile([128, NT], F32, tag="ps", name=f"psA{ot}_{b}")
                         for b in range(BG)]
                        for ot in range(3)
                    ]
                    for kb in range(0, KT, 2):
                        # weight-stream prefetch only from mid-blockA
                        # on, so it doesn't steal DMA bandwidth from
                        # the pipeline-filling x tiles
                        prefetch(1, 1 if kb >= KT // 2 else 0)
                        for ot in range(3):
                            mm_block(ps3[ot], ot, xq_cur, kb, kb + 2)
                    for ot in range(3):
                        epilogue(ps3[ot], g, ot)
                    ot_start = 3
                else:
                    ot_start = 0
                for ot in range(ot_start, OT):
                    prefetch(1, 1)
                    ps = [
                        psp.tile([128, NT], F32, tag="ps", name=f"ps{g}_{ot}_{b}")
                        for b in range(BG)
                    ]
                    if g == G - 1 and ot == OT - 1:
                        # Drain the final o-tile bank-major: bank 0's
                        # epilogue+store overlap bank 1's matmuls, so
                        # the post-last-matmul tail is one bank deep;
                        # the final bank splits its epilogue across
                        # VectorE and ScalarE with stores on both
                        # HWDGE rings so the two halves drain in
                        # parallel.
                        c = ot // OT_PER_CHUNK
                        for b in range(BG):
                            for k in range(KT):
                                nc.tensor.matmul(
                                    ps[b][:],
                                    lhsT=wq[:, ot, k],
                                    rhs=xq_cur[:, k, :, NT * b : NT * (b + 1)],
                                    start=(k == 0),
                                    stop=(k == KT - 1),
                                    perf_mode=DR,
                                )
                            ob = osbp.tile(
                                [128, NT], F32, tag="osb", name=f"ob{g}_{ot}_{b}"
                            )
                            col0 = TG * g + NT * b
                            if b < BG - 1:
                                nc.vector.tensor_scalar(
                                    ob[:],
                                    ps[b][:],
                                    al_b[c][:],
                                    bias_sb[:, ot : ot + 1],
                                    op0=mybir.AluOpType.mult,
                                    op1=mybir.AluOpType.add,
                                )
                                nc.sync.dma_start(
                                    out=outT[
                                        128 * ot : 128 * (ot + 1),
                                        col0 : col0 + NT,
                                    ],
                                    in_=ob[:],
                                )
                            else:
                                H = NT // 2
                                nc.vector.tensor_scalar(
                                    ob[:, :H],
                                    ps[b][:, :H],
                                    al_b[c][:],
                                    bias_sb[:, ot : ot + 1],
                                    op0=mybir.AluOpType.mult,
                                    op1=mybir.AluOpType.add,
                                )
                                nc.scalar.activation(
                                    ob[:, H:],
                                    ps[b][:, H:],
                                    mybir.ActivationFunctionType.Identity,
                                    bias=bias_sb[:, ot : ot + 1],
                                    scale=al_b[c][:],
                                )
                                nc.sync.dma_start(
                                    out=outT[
                                        128 * ot : 128 * (ot + 1),
                                        col0 : col0 + H,
                                    ],
                                    in_=ob[:, :H],
                                )
                                nc.scalar.dma_start(
                                    out=outT[
                                        128 * ot : 128 * (ot + 1),
                                        col0 + H : col0 + NT,
                                    ],
                                    in_=ob[:, H:],
                                )
                    else:
                        mm_block(ps, ot, xq_cur, 0, KT)
                        epilogue(ps, g, ot)
                prefetch(KT - nxt_k, OT - nxt_w)
                xq_cur = xq_next
    nc.compile()
    _CACHE[key] = nc
    return nc


def prepare_in_maps(x, w, bias, in_scale, w_scales):
    """Host-side prep: scale normalization, e4m3 quantization at half
    scale (bit-identical to the device ACT path it replaces), and
    layout permutation into the pre-tiled fp8 operand layouts."""
    assert x.shape == (B, S, IN) and w.shape == (OUT, IN)
    x = np.ascontiguousarray(x, dtype=np.float32)
    w = np.ascontiguousarray(w, dtype=np.float32)
    bias = np.ascontiguousarray(bias, dtype=np.float32)
    in_scale = np.float32(np.asarray(in_scale).reshape(()))
    w_scales = np.asarray(w_scales, dtype=np.float32).reshape(CHUNKS)

    half = np.float32(0.5)
    wn = (w.reshape(CHUNKS, OUT // CHUNKS, IN) / w_scales[:, None, None]).reshape(
        OUT, IN
    )
    w8 = (wn * half).astype(NP_FP8)
    # wq8[h, p, ot, k, ko, o'] = w8[o = OUT_C*h + 128*ot + o', i = 256*k + 128*ko + p]
    wq8_by_h = np.ascontiguousarray(
        w8.reshape(OUT_WAYS, OT, 128, KT, 2, 128).transpose(0, 5, 1, 3, 4, 2)
    )
    alpha_full = (
        4.0 * in_scale.astype(np.float64) * w_scales.astype(np.float64)
    ).astype(np.float32)

    x2d = x.reshape(TOK, IN)
    xq8_by_q = []
    for q in range(TOKEN_WAYS):
        xs = (x2d[T * q : T * (q + 1)] / in_scale * half).astype(NP_FP8)
        # xq8[p, g, k, ko, t] = xs[g*TG + t, i = 256*k + 128*ko + p]
        xq8_by_q.append(
            np.ascontiguousarray(
                xs.reshape(G, TG, KT, 2, 128).transpose(4, 0, 2, 3, 1)
            )
        )

    in_maps = []
    for cid in range(N_CORES):
        q, h = divmod(cid, OUT_WAYS)
        in_maps.append(
            {
                "xq8": xq8_by_q[q],
                "wq8": wq8_by_h[h],
                "biasv": bias[OUT_C * h : OUT_C * (h + 1)],
                "alpha": alpha_full[CHUNKS_C * h : CHUNKS_C * (h + 1)],
            }
        )
    return in_maps


def kernel(x, w, bias, in_scale, w_scales):
    nc = _build()
    in_maps = prepare_in_maps(x, w, bias, in_scale, w_scales)
    trace = bool(int(os.environ.get("TRN_KERNEL_TRACE", "0")))
    res = run_bass_kernel_spmd(nc, in_maps, list(range(N_CORES)), trace=trace)
    _CACHE["last_results"] = res

    out2d = np.empty((TOK, OUT), dtype=np.float32)
    for cid in range(N_CORES):
        q, h = divmod(cid, OUT_WAYS)
        out2d[T * q : T * (q + 1), OUT_C * h : OUT_C * (h + 1)] = res.results[cid][
            "outT"
        ].T
    return out2d.reshape(B, S, OUT)


# revision 22
# speedup vs baseline: 1.0043x; 1.0043x over previous
"""Bass/Trainium2 kernel for nn_DefaultSegmentLinear (fp8 segment linear).

Reference semantics (CHUNKS=4, seg_mode='weight'):
    xq = e4m3fn(x / in_scale)                       # OCP e4m3, max 448
    wq = e4m3fn(w_c / w_scales[c])                  # per out-chunk of 1024
    out = (xq @ wq_c^T) * in_scale * w_scales[c] + bias

Sharding: 4-way over the 16384 tokens x 2-way over the 4096 out
features (8 cores; core cid -> token quarter q=cid//2, out half
h=cid%2).

Quantization happens on the HOST: x and w are divided by their
calibration scales (exact f32 division, matching the reference), scaled
by 0.5, and rounded to TRN e4m3 (max 240) via ml_dtypes.float8_e4m3 --
every OCP-e4m3 grid point v <= 448 has v/2 <= 224 exactly representable
in TRN e4m3, and numpy's f32 multiply + RNE downcast is bit-identical
to the device ACT path the previous revision used.  The 4x is folded
into the output scale alpha_c = 4*in_scale*w_scales[c].  Shipping fp8
instead of f32 cuts device DMA-in 4x (x: 64->16 MB, w: 32->8 MB per
core) and removes the on-device quantize pass entirely.

Per-core DRAM tensors (contraction i on partitions for both operands):
    xq8  [128, G, KT, 2, TG] fp8   pre-tiled (x/in_scale/2)^T quarter,
         grouped into G=4 token groups of TG=1024 so matmuls start
         after ~one group's DMA instead of the full x load
    wq8  [128, OT, KT, 2, 128] fp8 pre-tiled (w/w_scale/2)^T half
    outT [OUT_C, T] f32  (o, t); host transposes back

Device schedule: wq stays resident in SBUF (64 KB/partition); xq token
groups double-buffer (2 x 32 KB/partition).  Per (group, o-tile): 16
k-steps x 2 token banks of DoubleRow fp8 matmuls (K=256, N=512) into
PSUM, then one DVE tensor_scalar (psum*alpha + bias) per bank and a DMA
out.  Startup latency is hidden three ways: throwaway warmup matmuls on
a memset tile keep the PE clock un-throttled through the input-DMA
window; group 0 runs a progressive-K schedule over its first 3 o-tiles
so the first real matmul waits on only ~1.25 MB of DMA; and the weight
stream + next-group x prefetches are paced through the two HWDGE rings
behind the startup-critical transfers.  The final o-tile drains
bank-major so the post-last-matmul tail is one bank deep.
"""

import os

import ml_dtypes
import numpy as np

import concourse.bacc as bacc
import concourse.mybir as mybir
from concourse import tile
from concourse.bass_utils import run_bass_kernel_spmd

N_CORES = 8
TOKEN_WAYS, OUT_WAYS = (
    int(v) for v in os.environ.get("TRN_KERNEL_SHARD", "4x2").split("x")
)
assert TOKEN_WAYS * OUT_WAYS == N_CORES
B, S, IN, OUT = 4, 4096, 4096, 4096
TOK = B * S
T = TOK // TOKEN_WAYS    # 4096 tokens per core
OUT_C = OUT // OUT_WAYS  # 2048 out features per core
KT = IN // 256           # 16 contraction super-tiles (256 = 128 x 2)
OT = OUT_C // 128        # 16 out-feature tiles per core
NT = int(os.environ.get("TRN_KERNEL_NT", "512"))  # free dim per matmul
TG = int(os.environ.get("TRN_KERNEL_TG", "1024"))  # tokens per group
G = T // TG              # token groups per core
BG = TG // NT            # PSUM tiles per (group, o-tile)
PS_BUFS = (8 * 512) // NT  # PSUM tile ring depth (8 banks total)
CHUNKS = 4
CHUNKS_C = CHUNKS // OUT_WAYS  # 2 weight chunks per core
OT_PER_CHUNK = OT // CHUNKS_C  # 8

F32 = mybir.dt.float32
FP8 = mybir.dt.float8e4
NP_FP8 = ml_dtypes.float8_e4m3

_CACHE = {}


def _build():
    key = ("nc", TG, NT)
    if key in _CACHE:
        return _CACHE[key]
    nc = bacc.Bacc(None, target_bir_lowering=False)
    xq8 = nc.dram_tensor("xq8", [128, G, KT, 2, TG], FP8, kind="ExternalInput")
    wq8 = nc.dram_tensor("wq8", [128, OT, KT, 2, 128], FP8, kind="ExternalInput")
    biasv = nc.dram_tensor("biasv", [OUT_C], F32, kind="ExternalInput")
    alpha = nc.dram_tensor("alpha", [CHUNKS_C], F32, kind="ExternalInput")
    outT = nc.dram_tensor("outT", [OUT_C, T], F32, kind="ExternalOutput")

    DR = mybir.MatmulPerfMode.DoubleRow

    with tile.TileContext(nc) as tc:
        with (
            tc.tile_pool(name="consts", bufs=1) as consts,
            tc.tile_pool(name="wq", bufs=1) as wqp,
            tc.tile_pool(name="xq", bufs=2) as xqp,
            tc.tile_pool(name="osb", bufs=4) as osbp,
            tc.tile_pool(name="psum", bufs=PS_BUFS, space="PSUM") as psp,
        ):
            wq = wqp.tile([128, OT, KT, 2, 128], FP8, tag="wq", name="wq")

            # Warm the PE clock (HAM un-throttles after ~3.4us of
            # sustained activity) with throwaway matmuls on a memset
            # tile -- no DMA dependency, so they start at ~0.3us and
            # span the whole input-DMA window; real matmuls then start
            # at 2.4 GHz instead of paying ~13 cold issues at 1.2 GHz.
            warm_w = consts.tile([128, 2, 128], FP8, tag="warm")
            nc.vector.memset(warm_w[:], 0.0)
            ps_warm = psp.tile([128, NT], F32, tag="ps", name="ps_warm")
            for _ in range(42):
                nc.tensor.matmul(
                    ps_warm[:, :128],
                    lhsT=warm_w[:],
                    rhs=warm_w[:],
                    start=True,
                    stop=True,
                    perf_mode=DR,
                )

            # DMA emission order controls which transfers the first
            # matmuls wait on: w[0] + group 0's first 8 k-tiles of x
            # (~2.5 MB) land first; everything else streams behind.
            # Descriptor generation costs ~600ns per dma_start on the
            # issuing sequencer, so the critical transfers are split
            # across both HWDGE rings (sync + scalar) to halve the
            # serial generation latency.
            xq_cur = xqp.tile([128, KT, 2, TG], FP8, tag="xq", name="xq0")
            nc.sync.dma_start(out=wq[:, 0], in_=wq8[:, 0])
            nc.scalar.dma_start(out=wq[:, 1], in_=wq8[:, 1])
            nc.sync.dma_start(out=xq_cur[:, 0], in_=xq8[:, 0, 0])
            nc.scalar.dma_start(out=xq_cur[:, 1], in_=xq8[:, 0, 1])
            nc.scalar.dma_start(out=wq[:, 2], in_=wq8[:, 2])
            for k in range(2, KT):
                eng = nc.sync if k % 2 == 0 else nc.scalar
                eng.dma_start(out=xq_cur[:, k], in_=xq8[:, 0, k])
            al_b = []
            for c in range(CHUNKS_C):
                t2 = consts.tile([128, 1], F32, tag=f"al{c}")
                nc.sync.dma_start(
                    out=t2[:], in_=alpha[c : c + 1].to_broadcast((128, 1))
                )
                al_b.append(t2)
            bias_sb = consts.tile([128, OT], F32, tag="bias")
            nc.sync.dma_start(
                out=bias_sb[:], in_=biasv[:].rearrange("(j p) -> p j", p=128)
            )

            def mm_block(ps, ot, xq_t, k_lo, k_hi):
                for k in range(k_lo, k_hi):
                    for b in range(BG):
                        nc.tensor.matmul(
                            ps[b][:],
                            lhsT=wq[:, ot, k],
                            rhs=xq_t[:, k, :, NT * b : NT * (b + 1)],
                            start=(k == 0),
                            stop=(k == KT - 1),
                            perf_mode=DR,
                        )

            def epilogue(ps, g, ot):
                c = ot // OT_PER_CHUNK
                for b in range(BG):
                    ob = osbp.tile([128, NT], F32, tag="osb", name=f"ob{g}_{ot}_{b}")
                    nc.vector.tensor_scalar(
                        ob[:],
                        ps[b][:],
                        al_b[c][:],
                        bias_sb[:, ot : ot + 1],
                        op0=mybir.AluOpType.mult,
                        op1=mybir.AluOpType.add,
                    )
                    nc.sync.dma_start(
                        out=outT[
                            128 * ot : 128 * (ot + 1),
                            TG * g + NT * b : TG * g + NT * (b + 1),
                        ],
                        in_=ob[:],
                    )

            for g in range(G):
                xq_next = None
                if g + 1 < G:
                    xq_next = xqp.tile(
                        [128, KT, 2, TG], FP8, tag="xq", name=f"xq{g + 1}"
                    )
                nxt_k = 0   # next k-tile of xq_next to prefetch
                nxt_w = 3   # next weight o-tile to prefetch (g0 only)

                def prefetch(n_x, n_w, g=g, xq_next=xq_next):
                    # The weight stream rides the sync HWDGE ring,
                    # where FIFO order behind the (epilogue-gated)
                    # output stores paces it -- otherwise the SDMA
                    # engines' fair round-robin lets these 6.5 MB
                    # steal bandwidth from the startup-critical
                    # transfers.  Next-group x rides the scalar ring.
                    nonlocal nxt_k, nxt_w
                    for _ in range(n_w):
                        if g == 0 and nxt_w < OT:
                            nc.sync.dma_start(out=wq[:, nxt_w], in_=wq8[:, nxt_w])
                            nxt_w += 1
                    for _ in range(n_x):
                        if xq_next is not None and nxt_k < KT:
                            nc.scalar.dma_start(
                                out=xq_next[:, nxt_k], in_=xq8[:, g + 1, nxt_k]
                            )
                            nxt_k += 1

                if g == 0:
                    # Progressive-K start: the first 3 o-tiles
                    # accumulate two k-tiles per pass, so the first
                    # matmul waits on only w[0..1] + x k[0..1]
                    # (~1.25 MB) and the rest of group 0's x streams
                    # in underneath the early (DMA-bound) passes.
                    ps3 = [
                        [psp.tile([128, NT], F32, tag="ps", name=f"psA{ot}_{b}")
                         for b in range(BG)]
                        for ot in range(3)
                    ]
                    for kb in range(0, KT, 2):
                        # weight-stream prefetch only from mid-blockA
                        # on, so it doesn't steal DMA bandwidth from
                        # the pipeline-filling x tiles
                        prefetch(1, 1 if kb >= KT // 2 else 0)
                        for ot in range(3):
                            mm_block(ps3[ot], ot, xq_cur, kb, kb + 2)
                    for ot in range(3):
                        epilogue(ps3[ot], g, ot)
                    ot_start = 3
                else:
                    ot_start = 0
                for ot in range(ot_start, OT):
                    prefetch(1, 1)
                    ps = [
                        psp.tile([128, NT], F32, tag="ps", name=f"ps{g}_{ot}_{b}")
                        for b in range(BG)
                    ]
                    if g == G - 1 and ot == OT - 1:
                        # Drain the final o-tile bank-major: bank 0's
                        # epilogue+store overlap bank 1's matmuls, so
                        # the post-last-matmul tail is one bank deep.
                        c = ot // OT_PER_CHUNK
                        for b in range(BG):
                            for k in range(KT):
                                nc.tensor.matmul(
                                    ps[b][:],
                                    lhsT=wq[:, ot, k],
                                    rhs=xq_cur[:, k, :, NT * b : NT * (b + 1)],
                                    start=(k == 0),
                                    stop=(k == KT - 1),
                                    perf_mode=DR,
                                )
                            ob = osbp.tile(
                                [128, NT], F32, tag="osb", name=f"ob{g}_{ot}_{b}"
                            )
                            nc.vector.tensor_scalar(
                                ob[:],
                                ps[b][:],
                                al_b[c][:],
                                bias_sb[:, ot : ot + 1],
                                op0=mybir.AluOpType.mult,
                                op1=mybir.AluOpType.add,
                            )
                            nc.sync.dma_start(
                                out=outT[
                                    128 * ot : 128 * (ot + 1),
                                    TG * g + NT * b : TG * g + NT * (b + 1),
                                ],
                                in_=ob[:],
                            )
                    else:
                        mm_block(ps, ot, xq_cur, 0, KT)
                        epilogue(ps, g, ot)
                prefetch(KT - nxt_k, OT - nxt_w)
                xq_cur = xq_next
    nc.compile()
    _CACHE[key] = nc
    return nc


def prepare_in_maps(x, w, bias, in_scale, w_scales):
    """Host-side prep: scale normalization, e4m3 quantization at half
    scale (bit-identical to the device ACT path it replaces), and
    layout permutation into the pre-tiled fp8 operand layouts."""
    assert x.shape == (B, S, IN) and w.shape == (OUT, IN)
    x = np.ascontiguousarray(x, dtype=np.float32)
    w = np.ascontiguousarray(w, dtype=np.float32)
    bias = np.ascontiguousarray(bias, dtype=np.float32)
    in_scale = np.float32(np.asarray(in_scale).reshape(()))
    w_scales = np.asarray(w_scales, dtype=np.float32).reshape(CHUNKS)

    half = np.float32(0.5)
    wn = (w.reshape(CHUNKS, OUT // CHUNKS, IN) / w_scales[:, None, None]).reshape(
        OUT, IN
    )
    w8 = (wn * half).astype(NP_FP8)
    # wq8[h, p, ot, k, ko, o'] = w8[o = OUT_C*h + 128*ot + o', i = 256*k + 128*ko + p]
    wq8_by_h = np.ascontiguousarray(
        w8.reshape(OUT_WAYS, OT, 128, KT, 2, 128).transpose(0, 5, 1, 3, 4, 2)
    )
    alpha_full = (
        4.0 * in_scale.astype(np.float64) * w_scales.astype(np.float64)
    ).astype(np.float32)

    x2d = x.reshape(TOK, IN)
    xq8_by_q = []
    for q in range(TOKEN_WAYS):
        xs = (x2d[T * q : T * (q + 1)] / in_scale * half).astype(NP_FP8)
        # xq8[p, g, k, ko, t] = xs[g*TG + t, i = 256*k + 128*ko + p]
        xq8_by_q.append(
            np.ascontiguousarray(
                xs.reshape(G, TG, KT, 2, 128).transpose(4, 0, 2, 3, 1)
            )
        )

    in_maps = []
    for cid in range(N_CORES):
        q, h = divmod(cid, OUT_WAYS)
        in_maps.append(
            {
                "xq8": xq8_by_q[q],
                "wq8": wq8_by_h[h],
                "biasv": bias[OUT_C * h : OUT_C * (h + 1)],
                "alpha": alpha_full[CHUNKS_C * h : CHUNKS_C * (h + 1)],
            }
        )
    return in_maps


def kernel(x, w, bias, in_scale, w_scales):
    nc = _build()
    in_maps = prepare_in_maps(x, w, bias, in_scale, w_scales)
    trace = bool(int(os.environ.get("TRN_KERNEL_TRACE", "0")))
    res = run_bass_kernel_spmd(nc, in_maps, list(range(N_CORES)), trace=trace)
    _CACHE["last_results"] = res

    out2d = np.empty((TOK, OUT), dtype=np.float32)
    for cid in range(N_CORES):
        q, h = divmod(cid, OUT_WAYS)
        out2d[T * q : T * (q + 1), OUT_C * h : OUT_C * (h + 1)] = res.results[cid][
            "outT"
        ].T
    return out2d.reshape(B, S, OUT)
